# revision 1
# baseline (speedup 1.0000x reference)
"""Sparse L1-distance attention (nn_L1AttnSparse) on 8 Trainium2 NeuronCores.

Sharding: dst tokens are split across the 8 cores (256 dst tokens each);
every core keeps the full k/v tables (8 MB each) in DRAM and uses the
custom SWDGE gather instruction (dma_gather / InstDMAGatherAnt) to pull
the 2 KB k/v rows for its edges.  Scores, softmax over the 32 slots and
the weighted v-sum run on DVE/ACT.  Batch index is folded into the gather
index (tables are [2*2048, 512]).
"""

import sys

sys.path.insert(0, "/opt/trn_rl_repo")

import numpy as np

import concourse.bass as bass
import concourse.tile as tile
from concourse import bacc, mybir
from concourse.bass_utils import run_bass_kernel_spmd

BS = 2
N_TOK = 2048
NH = 8
W = 64
S = 32  # dst_mxlen
HW = NH * W  # 512 floats per (b, tok) row
N_CORES = 8
DT = N_TOK // N_CORES  # dst tokens per core = 256
CHUNKS = DT // 128  # dst chunks of 128 per core = 2
SH = 4  # slot halves per chunk (gather granularity)
SLOTS_PER = S // SH  # 16
IDX_PER = 128 * SLOTS_PER  # 2048 indices per gather


def _wrap_idx(flat):
    """int16 index list -> [128, n/16] tile layout: idx i at [i%16, i//16],
    replicated down the 8 groups of 16 partitions."""
    n = flat.shape[0]
    w16 = np.zeros((16, n // 16), dtype=np.int16)
    w16[np.arange(n) % 16, np.arange(n) // 16] = flat
    return np.tile(w16, (8, 1))


def build_kernel():
    nc = bacc.Bacc(
        "TRN2", target_bir_lowering=False, debug=False, num_devices=N_CORES,
        dynamic_dma_scratch_size=16384 * 8,
    )
    f32 = mybir.dt.float32
    i16 = mybir.dt.int16

    kf = nc.dram_tensor("kf", [BS * N_TOK, HW], f32, kind="ExternalInput").ap()
    vf = nc.dram_tensor("vf", [BS * N_TOK, HW], f32, kind="ExternalInput").ap()
    qc = nc.dram_tensor("qc", [BS, CHUNKS, 128, HW], f32, kind="ExternalInput").ap()
    idx = nc.dram_tensor(
        "idx", [BS, CHUNKS, SH, 128, IDX_PER // 16], i16, kind="ExternalInput"
    ).ap()
    oc = nc.dram_tensor("oc", [BS, CHUNKS, 128, HW], f32, kind="ExternalOutput").ap()

    with tile.TileContext(nc) as tc:
        with (
            tc.tile_pool(name="big", bufs=4) as bigp,
            tc.tile_pool(name="small", bufs=3) as smp,
            tc.tile_pool(name="idxp", bufs=4) as idxp,
        ):
            for b in range(BS):
                for c in range(CHUNKS):
                    q_t = smp.tile([128, HW], f32, tag="q")
                    nc.sync.dma_start(out=q_t[:], in_=qc[b, c])
                    L = smp.tile([128, S * NH], f32, tag="L")
                    idx_ts = []
                    for sh in range(SH):
                        it = idxp.tile([128, IDX_PER // 16], i16, tag=f"idx{sh}")
                        nc.sync.dma_start(out=it[:], in_=idx[b, c, sh])
                        idx_ts.append(it)
                    for sh in range(SH):
                        kg = bigp.tile([128, SLOTS_PER, HW], f32, tag="g")
                        nc.gpsimd.dma_gather(
                            kg[:], kf, idx_ts[sh][:], IDX_PER, IDX_PER, HW,
                            queue_num=0,
                        )
                        # kg <- kg - q (broadcast q over the slot dim)
                        nc.vector.tensor_tensor(
                            out=kg[:],
                            in0=kg[:],
                            in1=q_t[:, None, :].to_broadcast([128, SLOTS_PER, HW]),
                            op=mybir.AluOpType.subtract,
                        )
                        # L[:, sh half] = sum_w |kg|   ([128, s*h])
                        nc.vector.tensor_reduce(
                            out=L[:, sh * SLOTS_PER * NH : (sh + 1) * SLOTS_PER * NH],
                            in_=kg[:].rearrange("p s (h w) -> p (s h) w", w=W),
                            axis=mybir.AxisListType.X,
                            op=mybir.AluOpType.add,
                            apply_absolute_value=True,
                        )
                    # --- softmax over s (strided views: L is [p, (s h)]) ---
                    Lv = L[:].rearrange("p (s h) -> p h s", h=NH)
                    Lmin = smp.tile([128, NH], f32, tag="lmin")
                    nc.vector.tensor_reduce(
                        out=Lmin[:], in_=Lv, axis=mybir.AxisListType.X,
                        op=mybir.AluOpType.min,
                    )
                    E = smp.tile([128, S * NH], f32, tag="E")
                    nc.vector.tensor_tensor(
                        out=E[:].rearrange("p (s h) -> p s h", h=NH),
                        in0=L[:].rearrange("p (s h) -> p s h", h=NH),
                        in1=Lmin[:, None, :].to_broadcast([128, S, NH]),
                        op=mybir.AluOpType.subtract,
                    )
                    nc.scalar.activation(
                        out=E[:], in_=E[:], func=mybir.ActivationFunctionType.Exp,
                        scale=-1.0 / np.sqrt(W),
                    )
                    den = smp.tile([128, NH], f32, tag="den")
                    nc.vector.tensor_reduce(
                        out=den[:],
                        in_=E[:].rearrange("p (s h) -> p h s", h=NH),
                        axis=mybir.AxisListType.X,
                        op=mybir.AluOpType.add,
                    )
                    rden = smp.tile([128, NH], f32, tag="rden")
                    nc.vector.reciprocal(rden[:], den[:])
                    Wt = smp.tile([128, S * NH], f32, tag="Wt")
                    nc.vector.tensor_tensor(
                        out=Wt[:].rearrange("p (s h) -> p s h", h=NH),
                        in0=E[:].rearrange("p (s h) -> p s h", h=NH),
                        in1=rden[:, None, :].to_broadcast([128, S, NH]),
                        op=mybir.AluOpType.mult,
                    )
                    # --- weighted v gather+sum ---
                    ot = None
                    for sh in range(SH):
                        vg = bigp.tile([128, SLOTS_PER, HW], f32, tag="g")
                        nc.gpsimd.dma_gather(
                            vg[:], vf, idx_ts[sh][:], IDX_PER, IDX_PER, HW,
                            queue_num=0,
                        )
                        wslice = Wt[:, sh * SLOTS_PER * NH : (sh + 1) * SLOTS_PER * NH]
                        nc.vector.tensor_tensor(
                            out=vg[:].rearrange("p s (h w) -> p s h w", w=W),
                            in0=vg[:].rearrange("p s (h w) -> p s h w", w=W),
                            in1=wslice.rearrange("p (s h) -> p s h", h=NH)[
                                :, :, :, None
                            ].to_broadcast([128, SLOTS_PER, NH, W]),
                            op=mybir.AluOpType.mult,
                        )
                        on = smp.tile([128, HW], f32, tag="on")
                        nc.vector.tensor_reduce(
                            out=on[:],
                            in_=vg[:].rearrange("p s hw -> p hw s"),
                            axis=mybir.AxisListType.X,
                            op=mybir.AluOpType.add,
                        )
                        if ot is None:
                            ot = on
                        else:
                            acc = smp.tile([128, HW], f32, tag="acc")
                            nc.vector.tensor_tensor(
                                out=acc[:], in0=ot[:], in1=on[:],
                                op=mybir.AluOpType.add,
                            )
                            ot = acc
                    nc.sync.dma_start(out=oc[b, c], in_=ot[:])
    nc.compile()
    return nc


_NC_CACHE = None
_LAST_IN_MAPS = None


def kernel(v, q, k, coo, dst_mxlen):
    global _NC_CACHE
    assert int(dst_mxlen) == S
    v = np.asarray(v, dtype=np.float32)
    q = np.asarray(q, dtype=np.float32)
    k = np.asarray(k, dtype=np.float32)
    coo = np.asarray(coo)

    # src table: srct[t, s] = src index of edge (dst=t, slot=s)
    srct = np.zeros((N_TOK, S), dtype=np.int64)
    srct[coo[:, 0], coo[:, 2]] = coo[:, 1]

    kf = k.reshape(BS * N_TOK, HW)
    vf = v.reshape(BS * N_TOK, HW)

    if _NC_CACHE is None:
        _NC_CACHE = build_kernel()
    nc = _NC_CACHE

    in_maps = []
    for core in range(N_CORES):
        lo = core * DT
        qc = q[:, lo : lo + DT].reshape(BS, CHUNKS, 128, HW)
        idx = np.zeros((BS, CHUNKS, SH, 128, IDX_PER // 16), dtype=np.int16)
        for b in range(BS):
            for c in range(CHUNKS):
                for sh in range(SH):
                    # index i = s_local*128 + p  ->  row b*2048 + srct[...]
                    sl = np.arange(SLOTS_PER) + sh * SLOTS_PER
                    flat = (
                        b * N_TOK
                        + srct[lo + c * 128 : lo + (c + 1) * 128, sl].T
                    ).reshape(-1).astype(np.int16)  # [s_local, p] -> flat
                    idx[b, c, sh] = _wrap_idx(flat)
        in_maps.append(
            {"kf": kf, "vf": vf, "qc": np.ascontiguousarray(qc), "idx": idx}
        )

    global _LAST_IN_MAPS
    _LAST_IN_MAPS = in_maps
    res = run_bass_kernel_spmd(nc, in_maps, list(range(N_CORES)))
    out = np.empty((BS, N_TOK, NH, W), dtype=np.float32)
    for core in range(N_CORES):
        lo = core * DT
        out[:, lo : lo + DT] = res.results[core]["oc"].reshape(BS, DT, NH, W)
    return out



# revision 7
# speedup vs baseline: 1.6911x; 1.6911x over previous
"""Sparse L1-distance attention (nn_L1AttnSparse) on 8 Trainium2 NeuronCores.

Layout strategy (v2): dst tokens are split across the 8 cores (256 each).
k/v tables are stored fp16 with a host-side feature permutation so that the
transpose-mode SWDGE gather (dma_gather transpose=True) lands features on
partitions with head h = partition//16 constant per partition.  The L1
score reduction over the 64 head features then becomes a PE matmul with a
constant 0/1 block mask (accumulated over the 4 column-chunks), softmax
needs no max-subtraction (scores <= 0; a constant bias keeps exp() in fp16
range, cancelled by the normalizer), and the weighted v-sum runs as fp16
tensor_tensor ops (2x DVE mode) with a tree reduction over slots.
"""

import sys

sys.path.insert(0, "/opt/trn_rl_repo")

import numpy as np

import concourse.bass as bass
import concourse.tile as tile
from concourse import bacc, mybir
from concourse.bass_utils import run_bass_kernel_spmd

BS = 2
N_TOK = 2048
NH = 8
W = 64
S = 32  # dst_mxlen
HW = NH * W  # 512 features per (b, tok) row
N_CORES = 8
DT = N_TOK // N_CORES  # dst tokens per core = 256
CHUNKS = DT // 128  # dst chunks of 128 per core = 2
NB = BS * CHUNKS  # blocks per core = 4
SHALF = S // 2  # slots per gather half = 16
EDGES_H = SHALF * 128  # edges per gather = 2048
C4 = HW // 128  # feature column-chunks in transpose-gather = 4
CEXP = 40.0  # constant score bias: exp((CEXP - L)/8), cancels in normalize
SCALE = 1.0 / np.sqrt(W)  # 1/8

# feature permutation: table column pos = c*128 + p holds original feature
# h*64 + c*16 + r where p = h*16 + r  ->  head h == p//16 for every c.
_P = np.arange(128)
_C = np.arange(C4)
COLPERM = (
    (_P[None, :] // 16) * 64 + _C[:, None] * 16 + (_P[None, :] % 16)
).reshape(-1)  # [pos] -> original feature index


def _wrap_idx(flat):
    """int16 index list -> [128, n/16] tile layout: idx i at [i%16, i//16],
    replicated down the 8 groups of 16 partitions."""
    n = flat.shape[0]
    w16 = np.zeros((16, n // 16), dtype=np.int16)
    w16[np.arange(n) % 16, np.arange(n) // 16] = flat
    return np.tile(w16, (8, 1))


def build_kernel():
    nc = bacc.Bacc(
        "TRN2", target_bir_lowering=False, debug=False, num_devices=N_CORES,
        dynamic_dma_scratch_size=32768, num_swdge_queues=4,
    )
    f16 = mybir.dt.float16
    f32 = mybir.dt.float32
    i16 = mybir.dt.int16

    kf = nc.dram_tensor("kf", [BS * N_TOK, HW], f16, kind="ExternalInput").ap()
    vf = nc.dram_tensor("vf", [BS * N_TOK, HW], f16, kind="ExternalInput").ap()
    qT = nc.dram_tensor("qT", [NB, 128, C4 * 128], f16, kind="ExternalInput").ap()
    msk = nc.dram_tensor("msk", [128, 128], f16, kind="ExternalInput").ap()
    idx = nc.dram_tensor(
        "idx", [NB, 2, 128, EDGES_H // 16], i16, kind="ExternalInput"
    ).ap()
    oc = nc.dram_tensor("oc", [NB, 128, C4 * 128], f16, kind="ExternalOutput").ap()

    with tile.TileContext(nc) as tc:
        with (
            nc.allow_low_precision(reason="fp16 datapath, fp32 score accum"),
            tc.tile_pool(name="big", bufs=2) as bigp,
            tc.tile_pool(name="small", bufs=2) as smp,
            tc.tile_pool(name="const", bufs=1) as cst,
            tc.tile_pool(name="psum", bufs=2, space="PSUM") as psp,
        ):
            msk_t = cst.tile([128, 128], f16, tag="msk")
            nc.sync.dma_start(out=msk_t[:], in_=msk)
            bias_t = cst.tile([128, 1], f32, tag="bias")
            nc.gpsimd.memset(bias_t[:], CEXP * SCALE)
            for blk in range(NB):
                idx_ts = []
                for hf in range(2):
                    it = smp.tile([128, EDGES_H // 16], i16, tag=f"idx{hf}")
                    nc.sync.dma_start(out=it[:], in_=idx[blk, hf])
                    idx_ts.append(it)
                qt = smp.tile([128, C4, 128], f16, tag="qt")
                nc.sync.dma_start(
                    out=qt[:], in_=qT[blk].rearrange("p (c d) -> p c d", c=C4)
                )
                E16 = smp.tile([128, S, 128], f16, tag="E")
                vgs = []
                for hf in range(2):
                    vg = bigp.tile([128, C4, EDGES_H], f16, tag=f"vg{hf}")
                    nc.gpsimd.dma_gather(
                        vg[:], vf, idx_ts[hf][:], EDGES_H, EDGES_H, HW,
                        transpose=True, queue_num=2 + hf,
                    )
                    vgs.append(vg)
                for hf in range(2):
                    kg = bigp.tile([128, C4, EDGES_H], f16, tag=f"kg{hf}")
                    nc.gpsimd.dma_gather(
                        kg[:], kf, idx_ts[hf][:], EDGES_H, EDGES_H, HW,
                        transpose=True, queue_num=hf,
                    )
                    kg4 = kg[:].rearrange("p c (s d) -> p c s d", d=128)
                    # kg <- kg - q (broadcast over slots); fp16 2x mode
                    nc.vector.tensor_tensor(
                        out=kg4, in0=kg4,
                        in1=qt[:, :, None, :].to_broadcast([128, C4, SHALF, 128]),
                        op=mybir.AluOpType.subtract,
                    )
                    # |diff| on the Activation engine
                    nc.scalar.activation(
                        out=kg[:], in_=kg[:], func=mybir.ActivationFunctionType.Abs
                    )
                    # score L via PE: psum[x, (s,d)] = sum_c sum_p msk[p,x]*|diff|
                    for sb in range(2):
                        ps = psp.tile([128, 8, 128], f32, tag="ps")
                        for half in range(2):
                            out_sl = ps[:, half * 4 : (half + 1) * 4, :]
                            s0 = sb * 8 + half * 4
                            for c in range(C4):
                                nc.tensor.matmul(
                                    out_sl, msk_t[:], kg4[:, c, s0 : s0 + 4, :],
                                    start=(c == 0), stop=(c == C4 - 1),
                                )
                        # E = exp((CEXP - L)/8), fp16, replicated over 16-groups
                        so = hf * SHALF + sb * 8
                        nc.scalar.activation(
                            out=E16[:, so : so + 8, :], in_=ps[:],
                            func=mybir.ActivationFunctionType.Exp,
                            scale=-SCALE, bias=bias_t[:],
                        )
                # denominator: tree-sum E over slots (fp16 TT adds, 2x mode)
                dtr = smp.tile([128, 16, 128], f16, tag="dtr")
                nc.vector.tensor_tensor(
                    out=dtr[:], in0=E16[:, :16, :], in1=E16[:, 16:, :],
                    op=mybir.AluOpType.add,
                )
                n = 8
                while n >= 2:
                    nc.vector.tensor_tensor(
                        out=dtr[:, :n, :], in0=dtr[:, :n, :],
                        in1=dtr[:, n : 2 * n, :],
                        op=mybir.AluOpType.add,
                    )
                    n //= 2
                den = smp.tile([128, 128], f32, tag="den")
                nc.vector.tensor_tensor(
                    out=den[:], in0=dtr[:, 0, :], in1=dtr[:, 1, :],
                    op=mybir.AluOpType.add,
                )
                rden32 = smp.tile([128, 128], f32, tag="rden32")
                nc.vector.reciprocal(rden32[:], den[:])
                rden = smp.tile([128, 128], f16, tag="rden")
                nc.vector.tensor_copy(rden[:], rden32[:])
                # weighted v: vg *= E (broadcast over c), then tree-sum slots
                for hf in range(2):
                    vg4 = vgs[hf][:].rearrange("p c (s d) -> p c s d", d=128)
                    nc.vector.tensor_tensor(
                        out=vg4, in0=vg4,
                        in1=E16[:, None, hf * SHALF : (hf + 1) * SHALF, :]
                        .to_broadcast([128, C4, SHALF, 128]),
                        op=mybir.AluOpType.mult,
                    )
                    n = 8
                    while n >= 1:
                        nc.vector.tensor_tensor(
                            out=vg4[:, :, :n, :], in0=vg4[:, :, :n, :],
                            in1=vg4[:, :, n : 2 * n, :],
                            op=mybir.AluOpType.add,
                        )
                        n //= 2
                vsum = smp.tile([128, C4, 128], f16, tag="vsum")
                nc.vector.tensor_tensor(
                    out=vsum[:],
                    in0=vgs[0][:].rearrange("p c (s d) -> p c s d", d=128)[:, :, 0, :],
                    in1=vgs[1][:].rearrange("p c (s d) -> p c s d", d=128)[:, :, 0, :],
                    op=mybir.AluOpType.add,
                )
                ot = smp.tile([128, C4, 128], f16, tag="ot")
                nc.vector.tensor_tensor(
                    out=ot[:], in0=vsum[:],
                    in1=rden[:, None, :].to_broadcast([128, C4, 128]),
                    op=mybir.AluOpType.mult,
                )
                nc.sync.dma_start(
                    out=oc[blk].rearrange("p (c d) -> p c d", c=C4), in_=ot[:]
                )
    nc.compile()
    return nc


_NC_CACHE = None


def kernel(v, q, k, coo, dst_mxlen):
    global _NC_CACHE
    assert int(dst_mxlen) == S
    v = np.asarray(v, dtype=np.float32)
    q = np.asarray(q, dtype=np.float32)
    k = np.asarray(k, dtype=np.float32)
    coo = np.asarray(coo)

    # src table: srct[t, s] = src index of edge (dst=t, slot=s)
    srct = np.zeros((N_TOK, S), dtype=np.int64)
    srct[coo[:, 0], coo[:, 2]] = coo[:, 1]

    kf = k.reshape(BS * N_TOK, HW)[:, COLPERM].astype(np.float16)
    vf = v.reshape(BS * N_TOK, HW)[:, COLPERM].astype(np.float16)
    q2 = q.reshape(BS, N_TOK, HW)[:, :, COLPERM].astype(np.float16)
    mskh = np.zeros((128, 128), dtype=np.float16)
    mskh[np.arange(128)[:, None] // 16 == np.arange(128)[None, :] // 16] = 1.0

    if _NC_CACHE is None:
        _NC_CACHE = build_kernel()
    nc = _NC_CACHE

    in_maps = []
    for core in range(N_CORES):
        lo0 = core * DT
        qTh = np.empty((NB, 128, C4 * 128), dtype=np.float16)
        idxh = np.empty((NB, 2, 128, EDGES_H // 16), dtype=np.int16)
        for b in range(BS):
            for c in range(CHUNKS):
                blk = b * CHUNKS + c
                lo = lo0 + c * 128
                # [d, pos] -> [p, c4, d]
                slab = q2[b, lo : lo + 128].reshape(128, C4, 128)
                qTh[blk] = slab.transpose(2, 1, 0).reshape(128, C4 * 128)
                for hf in range(2):
                    sl = slice(hf * SHALF, (hf + 1) * SHALF)
                    flat = (b * N_TOK + srct[lo : lo + 128, sl].T).reshape(-1)
                    idxh[blk, hf] = _wrap_idx(flat.astype(np.int16))
        in_maps.append(
            {"kf": kf, "vf": vf, "qT": qTh, "msk": mskh, "idx": idxh}
        )

    res = run_bass_kernel_spmd(nc, in_maps, list(range(N_CORES)))
    out = np.empty((BS, N_TOK, HW), dtype=np.float32)
    for core in range(N_CORES):
        lo0 = core * DT
        for b in range(BS):
            for c in range(CHUNKS):
                blk = b * CHUNKS + c
                lo = lo0 + c * 128
                o3 = res.results[core]["oc"][blk].reshape(128, C4, 128)
                out[b, lo : lo + 128, COLPERM] = (
                    o3.transpose(1, 0, 2).reshape(C4 * 128, 128)
                )
    return out.reshape(BS, N_TOK, NH, W)


# revision 8
# speedup vs baseline: 2.1968x; 1.2991x over previous
"""Sparse L1-distance attention (nn_L1AttnSparse) on 8 Trainium2 NeuronCores.

Layout strategy (v2): dst tokens are split across the 8 cores (256 each).
k/v tables are stored fp16 with a host-side feature permutation so that the
transpose-mode SWDGE gather (dma_gather transpose=True) lands features on
partitions with head h = partition//16 constant per partition.  The L1
score reduction over the 64 head features then becomes a PE matmul with a
constant 0/1 block mask (accumulated over the 4 column-chunks), softmax
needs no max-subtraction (scores <= 0; a constant bias keeps exp() in fp16
range, cancelled by the normalizer), and the weighted v-sum runs as fp16
tensor_tensor ops (2x DVE mode) with a tree reduction over slots.
"""

import sys

sys.path.insert(0, "/opt/trn_rl_repo")

import numpy as np

import concourse.bass as bass
import concourse.tile as tile
from concourse import bacc, mybir
from concourse.bass_utils import run_bass_kernel_spmd

BS = 2
N_TOK = 2048
NH = 8
W = 64
S = 32  # dst_mxlen
HW = NH * W  # 512 features per (b, tok) row
N_CORES = 8
DT = N_TOK // N_CORES  # dst tokens per core = 256
CHUNKS = DT // 128  # dst chunks of 128 per core = 2
NB = BS * CHUNKS  # blocks per core = 4
SHALF = S // 2  # slots per gather half = 16
EDGES_H = SHALF * 128  # edges per gather = 2048
C4 = HW // 128  # feature column-chunks in transpose-gather = 4
CEXP = 40.0  # constant score bias: exp((CEXP - L)/8), cancels in normalize
SCALE = 1.0 / np.sqrt(W)  # 1/8

# feature permutation: table column pos = c*128 + p holds original feature
# h*64 + c*16 + r where p = h*16 + r  ->  head h == p//16 for every c.
_P = np.arange(128)
_C = np.arange(C4)
COLPERM = (
    (_P[None, :] // 16) * 64 + _C[:, None] * 16 + (_P[None, :] % 16)
).reshape(-1)  # [pos] -> original feature index


def _wrap_idx(flat):
    """int16 index list -> [128, n/16] tile layout: idx i at [i%16, i//16],
    replicated down the 8 groups of 16 partitions."""
    n = flat.shape[0]
    w16 = np.zeros((16, n // 16), dtype=np.int16)
    w16[np.arange(n) % 16, np.arange(n) // 16] = flat
    return np.tile(w16, (8, 1))


def build_kernel():
    nc = bacc.Bacc(
        "TRN2", target_bir_lowering=False, debug=False, num_devices=N_CORES,
        dynamic_dma_scratch_size=32768, num_swdge_queues=4,
    )
    f16 = mybir.dt.float16
    f32 = mybir.dt.float32
    i16 = mybir.dt.int16

    kf = nc.dram_tensor("kf", [BS * N_TOK, HW], f16, kind="ExternalInput").ap()
    vf = nc.dram_tensor("vf", [BS * N_TOK, HW], f16, kind="ExternalInput").ap()
    qT = nc.dram_tensor("qT", [NB, 128, C4 * 128], f16, kind="ExternalInput").ap()
    msk = nc.dram_tensor("msk", [128, 128], f16, kind="ExternalInput").ap()
    idx = nc.dram_tensor(
        "idx", [NB, 2, 128, EDGES_H // 16], i16, kind="ExternalInput"
    ).ap()
    oc = nc.dram_tensor("oc", [NB, 128, C4 * 128], f16, kind="ExternalOutput").ap()

    with tile.TileContext(nc) as tc:
        with (
            nc.allow_low_precision(reason="fp16 datapath, fp32 score accum"),
            tc.tile_pool(name="big", bufs=2) as bigp,
            tc.tile_pool(name="small", bufs=2) as smp,
            tc.tile_pool(name="const", bufs=1) as cst,
            tc.tile_pool(name="psum", bufs=2, space="PSUM") as psp,
        ):
            msk_t = cst.tile([128, 128], f16, tag="msk")
            nc.sync.dma_start(out=msk_t[:], in_=msk)
            bias_t = cst.tile([128, 1], f32, tag="bias")
            nc.gpsimd.memset(bias_t[:], CEXP * SCALE)

            def make_inputs(blk):
                st = {}
                idx_ts = []
                for hf in range(2):
                    it = smp.tile([128, EDGES_H // 16], i16, tag=f"idx{hf}")
                    nc.sync.dma_start(out=it[:], in_=idx[blk, hf])
                    idx_ts.append(it)
                qt = smp.tile([128, C4, 128], f16, tag="qt")
                nc.sync.dma_start(
                    out=qt[:], in_=qT[blk].rearrange("p (c d) -> p c d", c=C4)
                )
                kgs, vgs = [], []
                for hf in range(2):
                    kg = bigp.tile([128, C4, EDGES_H], f16, tag=f"kg{hf}")
                    nc.gpsimd.dma_gather(
                        kg[:], kf, idx_ts[hf][:], EDGES_H, EDGES_H, HW,
                        transpose=True, queue_num=hf,
                    )
                    kgs.append(kg)
                for hf in range(2):
                    vg = bigp.tile([128, C4, EDGES_H], f16, tag=f"vg{hf}")
                    nc.gpsimd.dma_gather(
                        vg[:], vf, idx_ts[hf][:], EDGES_H, EDGES_H, HW,
                        transpose=True, queue_num=2 + hf,
                    )
                    vgs.append(vg)
                st["qt"], st["kgs"], st["vgs"] = qt, kgs, vgs
                return st

            def emit_score(blk, st):
                qt, kgs = st["qt"], st["kgs"]
                E16 = smp.tile([128, S, 128], f16, tag="E")
                st["E16"] = E16
                for hf in range(2):
                    kg4 = kgs[hf][:].rearrange("p c (s d) -> p c s d", d=128)
                    for sb in range(2):
                        ssl = slice(sb * 8, (sb + 1) * 8)
                        # kg <- kg - q (broadcast over slots); fp16 2x mode
                        nc.vector.tensor_tensor(
                            out=kg4[:, :, ssl, :], in0=kg4[:, :, ssl, :],
                            in1=qt[:, :, None, :].to_broadcast([128, C4, 8, 128]),
                            op=mybir.AluOpType.subtract,
                        )
                        # |diff| on the Activation engine
                        nc.scalar.activation(
                            out=kg4[:, :, ssl, :], in_=kg4[:, :, ssl, :],
                            func=mybir.ActivationFunctionType.Abs,
                        )
                        # L via PE: psum[x, (s,d)] = sum_c sum_p msk[p,x]*|diff|
                        ps = psp.tile([128, 8, 128], f32, tag="ps")
                        for half in range(2):
                            out_sl = ps[:, half * 4 : (half + 1) * 4, :]
                            s0 = sb * 8 + half * 4
                            for c in range(C4):
                                nc.tensor.matmul(
                                    out_sl, msk_t[:], kg4[:, c, s0 : s0 + 4, :],
                                    start=(c == 0), stop=(c == C4 - 1),
                                )
                        # E = exp((CEXP - L)/8), fp16, replicated over 16-groups
                        so = hf * SHALF + sb * 8
                        nc.scalar.activation(
                            out=E16[:, so : so + 8, :], in_=ps[:],
                            func=mybir.ActivationFunctionType.Exp,
                            scale=-SCALE, bias=bias_t[:],
                        )

            def emit_weight(blk, st):
                E16, vgs = st["E16"], st["vgs"]
                # denominator: tree-sum E over slots (fp16 TT adds, 2x mode)
                dtr = smp.tile([128, 16, 128], f16, tag="dtr")
                nc.vector.tensor_tensor(
                    out=dtr[:], in0=E16[:, :16, :], in1=E16[:, 16:, :],
                    op=mybir.AluOpType.add,
                )
                n = 8
                while n >= 2:
                    nc.vector.tensor_tensor(
                        out=dtr[:, :n, :], in0=dtr[:, :n, :],
                        in1=dtr[:, n : 2 * n, :],
                        op=mybir.AluOpType.add,
                    )
                    n //= 2
                den = smp.tile([128, 128], f32, tag="den")
                nc.vector.tensor_tensor(
                    out=den[:], in0=dtr[:, 0, :], in1=dtr[:, 1, :],
                    op=mybir.AluOpType.add,
                )
                rden = smp.tile([128, 128], f16, tag="rden")
                nc.vector.reciprocal(rden[:], den[:])
                # weighted v: vg *= E (broadcast over c), then tree-sum slots
                for hf in range(2):
                    vg4 = vgs[hf][:].rearrange("p c (s d) -> p c s d", d=128)
                    nc.vector.tensor_tensor(
                        out=vg4, in0=vg4,
                        in1=E16[:, None, hf * SHALF : (hf + 1) * SHALF, :]
                        .to_broadcast([128, C4, SHALF, 128]),
                        op=mybir.AluOpType.mult,
                    )
                    n = 8
                    while n >= 1:
                        nc.vector.tensor_tensor(
                            out=vg4[:, :, :n, :], in0=vg4[:, :, :n, :],
                            in1=vg4[:, :, n : 2 * n, :],
                            op=mybir.AluOpType.add,
                        )
                        n //= 2
                vsum = smp.tile([128, C4, 128], f16, tag="vsum")
                nc.vector.tensor_tensor(
                    out=vsum[:],
                    in0=vgs[0][:].rearrange("p c (s d) -> p c s d", d=128)[:, :, 0, :],
                    in1=vgs[1][:].rearrange("p c (s d) -> p c s d", d=128)[:, :, 0, :],
                    op=mybir.AluOpType.add,
                )
                ot = smp.tile([128, C4, 128], f16, tag="ot")
                nc.vector.tensor_tensor(
                    out=ot[:], in0=vsum[:],
                    in1=rden[:, None, :].to_broadcast([128, C4, 128]),
                    op=mybir.AluOpType.mult,
                )
                nc.sync.dma_start(
                    out=oc[blk].rearrange("p (c d) -> p c d", c=C4), in_=ot[:]
                )

            # software pipeline: block N+1's gathers + score phase are emitted
            # before block N's weighting phase so DVE fills the softmax latency
            pend = {0: make_inputs(0)}
            emit_score(0, pend[0])
            for blk in range(NB):
                if blk + 1 < NB:
                    pend[blk + 1] = make_inputs(blk + 1)
                    emit_score(blk + 1, pend[blk + 1])
                emit_weight(blk, pend.pop(blk))
    nc.compile()
    return nc


_NC_CACHE = None


def kernel(v, q, k, coo, dst_mxlen):
    global _NC_CACHE
    assert int(dst_mxlen) == S
    v = np.asarray(v, dtype=np.float32)
    q = np.asarray(q, dtype=np.float32)
    k = np.asarray(k, dtype=np.float32)
    coo = np.asarray(coo)

    # src table: srct[t, s] = src index of edge (dst=t, slot=s)
    srct = np.zeros((N_TOK, S), dtype=np.int64)
    srct[coo[:, 0], coo[:, 2]] = coo[:, 1]

    kf = k.reshape(BS * N_TOK, HW)[:, COLPERM].astype(np.float16)
    vf = v.reshape(BS * N_TOK, HW)[:, COLPERM].astype(np.float16)
    q2 = q.reshape(BS, N_TOK, HW)[:, :, COLPERM].astype(np.float16)
    mskh = np.zeros((128, 128), dtype=np.float16)
    mskh[np.arange(128)[:, None] // 16 == np.arange(128)[None, :] // 16] = 1.0

    if _NC_CACHE is None:
        _NC_CACHE = build_kernel()
    nc = _NC_CACHE

    in_maps = []
    for core in range(N_CORES):
        lo0 = core * DT
        qTh = np.empty((NB, 128, C4 * 128), dtype=np.float16)
        idxh = np.empty((NB, 2, 128, EDGES_H // 16), dtype=np.int16)
        for b in range(BS):
            for c in range(CHUNKS):
                blk = b * CHUNKS + c
                lo = lo0 + c * 128
                # [d, pos] -> [p, c4, d]
                slab = q2[b, lo : lo + 128].reshape(128, C4, 128)
                qTh[blk] = slab.transpose(2, 1, 0).reshape(128, C4 * 128)
                for hf in range(2):
                    sl = slice(hf * SHALF, (hf + 1) * SHALF)
                    flat = (b * N_TOK + srct[lo : lo + 128, sl].T).reshape(-1)
                    idxh[blk, hf] = _wrap_idx(flat.astype(np.int16))
        in_maps.append(
            {"kf": kf, "vf": vf, "qT": qTh, "msk": mskh, "idx": idxh}
        )

    res = run_bass_kernel_spmd(nc, in_maps, list(range(N_CORES)))
    out = np.empty((BS, N_TOK, HW), dtype=np.float32)
    for core in range(N_CORES):
        lo0 = core * DT
        for b in range(BS):
            for c in range(CHUNKS):
                blk = b * CHUNKS + c
                lo = lo0 + c * 128
                o3 = res.results[core]["oc"][blk].reshape(128, C4, 128)
                out[b, lo : lo + 128, COLPERM] = (
                    o3.transpose(1, 0, 2).reshape(C4 * 128, 128)
                )
    return out.reshape(BS, N_TOK, NH, W)


# revision 10
# speedup vs baseline: 2.2551x; 1.0265x over previous
"""Sparse L1-distance attention (nn_L1AttnSparse) on 8 Trainium2 NeuronCores.

Layout strategy (v2): dst tokens are split across the 8 cores (256 each).
k/v tables are stored fp16 with a host-side feature permutation so that the
transpose-mode SWDGE gather (dma_gather transpose=True) lands features on
partitions with head h = partition//16 constant per partition.  The L1
score reduction over the 64 head features then becomes a PE matmul with a
constant 0/1 block mask (accumulated over the 4 column-chunks), softmax
needs no max-subtraction (scores <= 0; a constant bias keeps exp() in fp16
range, cancelled by the normalizer), and the weighted v-sum runs as fp16
tensor_tensor ops (2x DVE mode) with a tree reduction over slots.
"""

import sys

sys.path.insert(0, "/opt/trn_rl_repo")

import numpy as np

import concourse.bass as bass
import concourse.tile as tile
from concourse import bacc, mybir
from concourse.bass_utils import run_bass_kernel_spmd

BS = 2
N_TOK = 2048
NH = 8
W = 64
S = 32  # dst_mxlen
HW = NH * W  # 512 features per (b, tok) row
N_CORES = 8
DT = N_TOK // N_CORES  # dst tokens per core = 256
CHUNKS = DT // 128  # dst chunks of 128 per core = 2
NB = BS * CHUNKS  # blocks per core = 4
SHALF = S // 2  # slots per gather half = 16
EDGES_H = SHALF * 128  # edges per gather = 2048
C4 = HW // 128  # feature column-chunks in transpose-gather = 4
CEXP = 40.0  # constant score bias: exp((CEXP - L)/8), cancels in normalize
SCALE = 1.0 / np.sqrt(W)  # 1/8

# feature permutation: table column pos = c*128 + p holds original feature
# h*64 + c*16 + r where p = h*16 + r  ->  head h == p//16 for every c.
_P = np.arange(128)
_C = np.arange(C4)
COLPERM = (
    (_P[None, :] // 16) * 64 + _C[:, None] * 16 + (_P[None, :] % 16)
).reshape(-1)  # [pos] -> original feature index


def _wrap_idx(flat):
    """int16 index list -> [128, n/16] tile layout: idx i at [i%16, i//16],
    replicated down the 8 groups of 16 partitions."""
    n = flat.shape[0]
    w16 = np.zeros((16, n // 16), dtype=np.int16)
    w16[np.arange(n) % 16, np.arange(n) // 16] = flat
    return np.tile(w16, (8, 1))


def build_kernel():
    nc = bacc.Bacc(
        "TRN2", target_bir_lowering=False, debug=False, num_devices=N_CORES,
        dynamic_dma_scratch_size=32768, num_swdge_queues=4,
    )
    f16 = mybir.dt.float16
    f32 = mybir.dt.float32
    i16 = mybir.dt.int16

    kf = nc.dram_tensor("kf", [BS * N_TOK, HW], f16, kind="ExternalInput").ap()
    vf = nc.dram_tensor("vf", [BS * N_TOK, HW], f16, kind="ExternalInput").ap()
    qT = nc.dram_tensor("qT", [NB, 128, C4 * 128], f16, kind="ExternalInput").ap()
    msk = nc.dram_tensor("msk", [128, 128], f16, kind="ExternalInput").ap()
    idx = nc.dram_tensor(
        "idx", [NB, 2, 128, EDGES_H // 16], i16, kind="ExternalInput"
    ).ap()
    oc = nc.dram_tensor("oc", [NB, 128, C4 * 128], f16, kind="ExternalOutput").ap()

    with tile.TileContext(nc) as tc:
        with (
            nc.allow_low_precision(reason="fp16 datapath, fp32 score accum"),
            tc.tile_pool(name="big", bufs=2) as bigp,
            tc.tile_pool(name="small", bufs=2) as smp,
            tc.tile_pool(name="const", bufs=1) as cst,
            tc.tile_pool(name="psum", bufs=2, space="PSUM") as psp,
        ):
            msk_t = cst.tile([128, 128], f16, tag="msk")
            bias_t = cst.tile([128, 1], f32, tag="bias")
            nc.gpsimd.memset(bias_t[:], CEXP * SCALE)

            def make_inputs(blk):
                st = {}
                idx_ts = []
                for hf in range(2):
                    it = smp.tile([128, EDGES_H // 16], i16, tag=f"idx{hf}")
                    nc.sync.dma_start(out=it[:], in_=idx[blk, hf])
                    idx_ts.append(it)
                # k gathered in slot-block quarters (1024 rows) so the score
                # pipeline can start on the first quarter early
                kgs = []
                for qq in range(4):
                    kg = bigp.tile([128, C4, EDGES_H // 2], f16, tag=f"kg{qq}")
                    it = idx_ts[qq // 2]
                    nc.gpsimd.dma_gather(
                        kg[:], kf, it[:, (qq % 2) * 64 : (qq % 2 + 1) * 64],
                        EDGES_H // 2, EDGES_H // 2, HW,
                        transpose=True, queue_num=qq % 2,
                    )
                    kgs.append(kg)
                vgs = []
                for hf in range(2):
                    vg = bigp.tile([128, C4, EDGES_H], f16, tag=f"vg{hf}")
                    nc.gpsimd.dma_gather(
                        vg[:], vf, idx_ts[hf][:], EDGES_H, EDGES_H, HW,
                        transpose=True, queue_num=2 + hf,
                    )
                    vgs.append(vg)
                qt = smp.tile([128, C4, 128], f16, tag="qt")
                nc.sync.dma_start(
                    out=qt[:], in_=qT[blk].rearrange("p (c d) -> p c d", c=C4)
                )
                st["qt"], st["kgs"], st["vgs"] = qt, kgs, vgs
                return st

            def emit_score(blk, st):
                qt, kgs = st["qt"], st["kgs"]
                E16 = smp.tile([128, S, 128], f16, tag="E")
                st["E16"] = E16
                for qq in range(4):
                    kg4 = kgs[qq][:].rearrange("p c (s d) -> p c s d", d=128)
                    # kg <- kg - q (broadcast over slots); fp16 2x mode
                    nc.vector.tensor_tensor(
                        out=kg4, in0=kg4,
                        in1=qt[:, :, None, :].to_broadcast([128, C4, 8, 128]),
                        op=mybir.AluOpType.subtract,
                    )
                    # |diff| on the Activation engine
                    nc.scalar.activation(
                        out=kg4, in_=kg4,
                        func=mybir.ActivationFunctionType.Abs,
                    )
                    # L via PE: psum[x, (s,d)] = sum_c sum_p msk[p,x]*|diff|
                    ps = psp.tile([128, 8, 128], f32, tag="ps")
                    for half in range(2):
                        out_sl = ps[:, half * 4 : (half + 1) * 4, :]
                        s0 = half * 4
                        for c in range(C4):
                            nc.tensor.matmul(
                                out_sl, msk_t[:], kg4[:, c, s0 : s0 + 4, :],
                                start=(c == 0), stop=(c == C4 - 1),
                            )
                    # E = exp((CEXP - L)/8), fp16, replicated over 16-groups
                    nc.scalar.activation(
                        out=E16[:, qq * 8 : (qq + 1) * 8, :], in_=ps[:],
                        func=mybir.ActivationFunctionType.Exp,
                        scale=-SCALE, bias=bias_t[:],
                    )

            def emit_weight(blk, st):
                E16, vgs = st["E16"], st["vgs"]
                # denominator: tree-sum E over slots (fp16 TT adds, 2x mode)
                dtr = smp.tile([128, 16, 128], f16, tag="dtr")
                nc.vector.tensor_tensor(
                    out=dtr[:], in0=E16[:, :16, :], in1=E16[:, 16:, :],
                    op=mybir.AluOpType.add,
                )
                n = 8
                while n >= 2:
                    nc.vector.tensor_tensor(
                        out=dtr[:, :n, :], in0=dtr[:, :n, :],
                        in1=dtr[:, n : 2 * n, :],
                        op=mybir.AluOpType.add,
                    )
                    n //= 2
                den = smp.tile([128, 128], f32, tag="den")
                nc.vector.tensor_tensor(
                    out=den[:], in0=dtr[:, 0, :], in1=dtr[:, 1, :],
                    op=mybir.AluOpType.add,
                )
                rden = smp.tile([128, 128], f16, tag="rden")
                nc.vector.reciprocal(rden[:], den[:])
                # weighted v: vg *= E (broadcast over c), then tree-sum slots
                for hf in range(2):
                    vg4 = vgs[hf][:].rearrange("p c (s d) -> p c s d", d=128)
                    nc.vector.tensor_tensor(
                        out=vg4, in0=vg4,
                        in1=E16[:, None, hf * SHALF : (hf + 1) * SHALF, :]
                        .to_broadcast([128, C4, SHALF, 128]),
                        op=mybir.AluOpType.mult,
                    )
                    n = 8
                    while n >= 1:
                        nc.vector.tensor_tensor(
                            out=vg4[:, :, :n, :], in0=vg4[:, :, :n, :],
                            in1=vg4[:, :, n : 2 * n, :],
                            op=mybir.AluOpType.add,
                        )
                        n //= 2
                vsum = smp.tile([128, C4, 128], f16, tag="vsum")
                nc.vector.tensor_tensor(
                    out=vsum[:],
                    in0=vgs[0][:].rearrange("p c (s d) -> p c s d", d=128)[:, :, 0, :],
                    in1=vgs[1][:].rearrange("p c (s d) -> p c s d", d=128)[:, :, 0, :],
                    op=mybir.AluOpType.add,
                )
                ot = smp.tile([128, C4, 128], f16, tag="ot")
                nc.vector.tensor_tensor(
                    out=ot[:], in0=vsum[:],
                    in1=rden[:, None, :].to_broadcast([128, C4, 128]),
                    op=mybir.AluOpType.mult,
                )
                # store on the ACT engine's DGE so SP's in-order queue never
                # delays the next block's idx/q loads behind this store
                nc.scalar.dma_start(
                    out=oc[blk].rearrange("p (c d) -> p c d", c=C4), in_=ot[:]
                )

            # software pipeline: block N+1's gathers + score phase are emitted
            # before block N's weighting phase so DVE fills the softmax latency
            pend = {0: make_inputs(0)}
            nc.sync.dma_start(out=msk_t[:], in_=msk)
            emit_score(0, pend[0])
            for blk in range(NB):
                if blk + 1 < NB:
                    pend[blk + 1] = make_inputs(blk + 1)
                    emit_score(blk + 1, pend[blk + 1])
                emit_weight(blk, pend.pop(blk))
    nc.compile()
    return nc


_NC_CACHE = None


def kernel(v, q, k, coo, dst_mxlen):
    global _NC_CACHE
    assert int(dst_mxlen) == S
    v = np.asarray(v, dtype=np.float32)
    q = np.asarray(q, dtype=np.float32)
    k = np.asarray(k, dtype=np.float32)
    coo = np.asarray(coo)

    # src table: srct[t, s] = src index of edge (dst=t, slot=s)
    srct = np.zeros((N_TOK, S), dtype=np.int64)
    srct[coo[:, 0], coo[:, 2]] = coo[:, 1]

    kf = k.reshape(BS * N_TOK, HW)[:, COLPERM].astype(np.float16)
    vf = v.reshape(BS * N_TOK, HW)[:, COLPERM].astype(np.float16)
    q2 = q.reshape(BS, N_TOK, HW)[:, :, COLPERM].astype(np.float16)
    mskh = np.zeros((128, 128), dtype=np.float16)
    mskh[np.arange(128)[:, None] // 16 == np.arange(128)[None, :] // 16] = 1.0

    if _NC_CACHE is None:
        _NC_CACHE = build_kernel()
    nc = _NC_CACHE

    in_maps = []
    for core in range(N_CORES):
        lo0 = core * DT
        qTh = np.empty((NB, 128, C4 * 128), dtype=np.float16)
        idxh = np.empty((NB, 2, 128, EDGES_H // 16), dtype=np.int16)
        for b in range(BS):
            for c in range(CHUNKS):
                blk = b * CHUNKS + c
                lo = lo0 + c * 128
                # [d, pos] -> [p, c4, d]
                slab = q2[b, lo : lo + 128].reshape(128, C4, 128)
                qTh[blk] = slab.transpose(2, 1, 0).reshape(128, C4 * 128)
                for hf in range(2):
                    sl = slice(hf * SHALF, (hf + 1) * SHALF)
                    flat = (b * N_TOK + srct[lo : lo + 128, sl].T).reshape(-1)
                    idxh[blk, hf] = _wrap_idx(flat.astype(np.int16))
        in_maps.append(
            {"kf": kf, "vf": vf, "qT": qTh, "msk": mskh, "idx": idxh}
        )

    res = run_bass_kernel_spmd(nc, in_maps, list(range(N_CORES)))
    out = np.empty((BS, N_TOK, HW), dtype=np.float32)
    for core in range(N_CORES):
        lo0 = core * DT
        for b in range(BS):
            for c in range(CHUNKS):
                blk = b * CHUNKS + c
                lo = lo0 + c * 128
                o3 = res.results[core]["oc"][blk].reshape(128, C4, 128)
                out[b, lo : lo + 128, COLPERM] = (
                    o3.transpose(1, 0, 2).reshape(C4 * 128, 128)
                )
    return out.reshape(BS, N_TOK, NH, W)
